# revision 16
# baseline (speedup 1.0000x reference)
"""Two-layer GCN encoder (GCNConv -> BatchNorm -> ELU -> GCNConv) on 8 trn2
NeuronCores.

Sharding: nodes are partitioned across the 8 cores (graph/data parallel).
Each core computes g1 = (x_local @ W1) * dinv (rowwise), the scaled feature
table is AllGathered, and each core aggregates messages for its own
destination nodes by gathering source rows with dma_gather (round-robin over
the 4 SWDGE queues so descriptor generation parallelizes across Q7 core
pairs) and scatter-adding them into PSUM via selection-matrix matmuls.

The selection matrices (one-hot over destination rows, with the dst-side
normalization dinv[d] folded into the nonzeros) are precomputed on the host
and streamed from DRAM via HWDGE - no per-block DVE/ACT work, and no DVE<->
GpSimd SBUF port contention.

Layer 1 aggregates with the selection matrix as lhsT (output [dst, feat]);
BN statistics are AllReduced.  Layer 2 aggregates transposed (selection
matrix as rhs, gathered features as lhsT, output [feat, dst]) so the final
W2 matmul needs no per-tile transpose: z = A2^T @ W2 with b2 folded in as a
rank-1 matmul.

Self-loops are applied as a diag(dinv) matmul per destination tile (the
local table rows are resident in SBUF and already carry one dinv factor).
"""

import numpy as np
import ml_dtypes

import concourse.bass as bass
import concourse.bacc as bacc
import concourse.mybir as mybir
import concourse.tile as tile
from concourse.alu_op_type import AluOpType
from concourse import library_config
from concourse.bass_utils import run_bass_kernel_spmd

P = 128
M = 8  # cores
NQ = 4  # SWDGE gather queues
BF16 = ml_dtypes.bfloat16
AF = mybir.ActivationFunctionType


# --------------------------------------------------------------------------
# Host-side preprocessing
# --------------------------------------------------------------------------

def preprocess(edge_index, n, gt_tiles=4):
    """Sort/partition edges, build per-core padded index arrays, the dense
    selection matrices, and the (uniform across cores) block structure."""
    src = np.asarray(edge_index[0], dtype=np.int64)
    dst = np.asarray(edge_index[1], dtype=np.int64)
    e = src.shape[0]

    n_per = n // M
    assert n_per * M == n
    T = (n_per + P - 1) // P
    ncp = T * P                      # padded nodes per core
    R = M * ncp                      # rows in the gathered table
    Rh = R // 2                      # half split (must fit int16)
    assert Rh < 32768, f"half table {Rh} rows exceeds int16 range"
    assert (4 * ncp) == Rh           # half boundary aligns with core boundary

    deg = np.bincount(dst, minlength=n).astype(np.float64) + 1.0
    dinv = (deg ** -0.5).astype(np.float32)

    # padded global row of each source node
    owner = src // n_per
    r_src = owner * ncp + (src - owner * n_per)          # [e]
    half = (r_src >= Rh).astype(np.int64)
    idx16 = (r_src - half * Rh).astype(np.int64)

    core_of = dst // n_per
    l_dst = dst - core_of * n_per
    t_dst = l_dst // P
    dstloc = l_dst % P

    # per (core, tile, half) edge counts -> uniform block counts
    counts = np.zeros((M, T, 2), dtype=np.int64)
    np.add.at(counts, (core_of, t_dst, half), 1)
    Bth = np.ceil(counts.max(axis=0) / P).astype(np.int64)   # [T, 2]

    # groups of tiles for gathers
    groups = []
    off_lo = off_hi = 0
    for g0 in range(0, T, gt_tiles):
        tiles = list(range(g0, min(g0 + gt_tiles, T)))
        nb_lo = int(sum(Bth[t, 0] for t in tiles))
        nb_hi = int(sum(Bth[t, 1] for t in tiles))
        groups.append(dict(tiles=tiles, off_lo=off_lo, nb_lo=nb_lo,
                           off_hi=off_hi, nb_hi=nb_hi))
        off_lo += nb_lo
        off_hi += nb_hi
    NB_lo, NB_hi = off_lo, off_hi

    # per-tile block index lists (global block number within its half array)
    tile_blocks = []   # [T] -> (lo_block_ids, hi_block_ids)
    blo = bhi = 0
    for t in range(T):
        lo_ids = list(range(blo, blo + int(Bth[t, 0]))); blo += int(Bth[t, 0])
        hi_ids = list(range(bhi, bhi + int(Bth[t, 1]))); bhi += int(Bth[t, 1])
        tile_blocks.append((lo_ids, hi_ids))

    meta = dict(n=n, e=e, n_per=n_per, T=T, ncp=ncp, R=R, Rh=Rh,
                Bth=Bth, groups=groups, tile_blocks=tile_blocks,
                NB_lo=NB_lo, NB_hi=NB_hi)

    # --- per-core arrays ---
    order = np.lexsort((half, t_dst, core_of))   # sort by core, tile, half
    src_s = idx16[order]
    dl_s = dstloc[order]
    dv_s = dinv[dst[order]]          # dinv of the (global) destination

    per_core = []
    flat_counts = counts  # [M,T,2]
    run_start = np.zeros((M, T, 2), dtype=np.int64)
    np.cumsum(flat_counts.reshape(-1), out=run_start.reshape(-1))
    run_start = run_start - flat_counts  # exclusive prefix

    for c in range(M):
        idx_lo = np.zeros(max(NB_lo, 1) * P, dtype=np.int16)
        idx_hi = np.zeros(max(NB_hi, 1) * P, dtype=np.int16)
        # dense selection matrices, flat over slots
        st_lo = np.zeros((max(NB_lo, 1) * P, P), dtype=np.float32)
        st_hi = np.zeros((max(NB_hi, 1) * P, P), dtype=np.float32)
        for t in range(T):
            for h, (idx_arr, st_arr, blk_ids) in enumerate(
                    ((idx_lo, st_lo, tile_blocks[t][0]),
                     (idx_hi, st_hi, tile_blocks[t][1]))):
                cnt = int(flat_counts[c, t, h])
                s0 = int(run_start[c, t, h])
                if not blk_ids:
                    assert cnt == 0
                    continue
                dst_off = blk_ids[0] * P
                idx_arr[dst_off:dst_off + cnt] = src_s[s0:s0 + cnt]
                slots = np.arange(dst_off, dst_off + cnt)
                st_arr[slots, dl_s[s0:s0 + cnt]] = dv_s[s0:s0 + cnt]
                assert cnt <= len(blk_ids) * P

        def wrap16(a):
            w = a.reshape(-1, 16).T.copy()          # [16, L/16]
            return np.tile(w, (8, 1))               # [128, L/16]

        def part_first(st, nb):
            # [nb*P, P] -> [P(slot), nb, P(dst)]
            return np.ascontiguousarray(
                st.reshape(nb, P, P).transpose(1, 0, 2)).astype(BF16)

        per_core.append(dict(
            idx_lo=wrap16(idx_lo), idx_hi=wrap16(idx_hi),
            st_lo=part_first(st_lo, max(NB_lo, 1)),
            st_hi=part_first(st_hi, max(NB_hi, 1)),
        ))

    return meta, per_core, dinv


# --------------------------------------------------------------------------
# Bass program
# --------------------------------------------------------------------------

def build_program(meta, c_in, c_hid, c_out, bn_eps=1e-5):
    n, T, ncp, R, Rh = meta["n"], meta["T"], meta["ncp"], meta["R"], meta["Rh"]
    n_per = meta["n_per"]
    NB_lo, NB_hi = meta["NB_lo"], meta["NB_hi"]
    groups = meta["groups"]
    tile_blocks = meta["tile_blocks"]
    assert c_in == P and c_hid == P

    f32 = mybir.dt.float32
    bf16 = mybir.dt.bfloat16
    i16 = mybir.dt.int16

    nc = bacc.Bacc(None, target_bir_lowering=False, debug=False, num_devices=M,
                   num_swdge_queues=NQ)

    # ---- I/O ----
    xT_d = nc.declare_dram_parameter("xT", [c_in, ncp], bf16, isOutput=False)
    W1_d = nc.declare_dram_parameter("W1b", [c_in, c_hid], bf16, isOutput=False)
    W2_d = nc.declare_dram_parameter("W2b", [c_hid, c_out], bf16, isOutput=False)
    dinv_d = nc.declare_dram_parameter("dinv_t", [P, T], f32, isOutput=False)
    ddiag_d = nc.declare_dram_parameter("ddiag", [P, T, P], bf16, isOutput=False)
    gamma_d = nc.declare_dram_parameter("gamma_r", [1, c_hid], f32, isOutput=False)
    beta_d = nc.declare_dram_parameter("beta_r", [1, c_hid], f32, isOutput=False)
    b2_d = nc.declare_dram_parameter("b2_r", [1, c_out], bf16, isOutput=False)
    onesc_d = nc.declare_dram_parameter("ones_col", [P, 1], bf16, isOutput=False)
    onescf_d = nc.declare_dram_parameter("ones_col_f32", [P, 1], f32, isOutput=False)
    onesr_d = nc.declare_dram_parameter("ones_row", [1, P], bf16, isOutput=False)
    idxlo_d = nc.declare_dram_parameter("idx_lo", [P, max(NB_lo, 1) * 8], i16, isOutput=False)
    idxhi_d = nc.declare_dram_parameter("idx_hi", [P, max(NB_hi, 1) * 8], i16, isOutput=False)
    stlo_d = nc.declare_dram_parameter("st_lo", [P, max(NB_lo, 1), P], bf16, isOutput=False)
    sthi_d = nc.declare_dram_parameter("st_hi", [P, max(NB_hi, 1), P], bf16, isOutput=False)
    zout_d = nc.declare_dram_parameter("zout", [n_per, c_out], f32, isOutput=True)

    nc.gpsimd.load_library(library_config.mlp)

    with tile.TileContext(nc) as tc:
        with (
            tc.tile_pool(name="dram", bufs=1, space="DRAM") as dram,
            tc.tile_pool(name="singles", bufs=1) as singles,
            tc.tile_pool(name="gather", bufs=12) as gpool,
            tc.tile_pool(name="stp", bufs=3) as stpool,
            tc.tile_pool(name="scratch", bufs=2) as scratch,
            tc.tile_pool(name="psum", bufs=3, space="PSUM") as psum,
            tc.tile_pool(name="psumaux", bufs=2, space="PSUM") as psumaux,
            tc.tile_pool(name="psum1", bufs=1, space="PSUM") as psum1,
        ):
            # ---- persistent SBUF ----
            xT_s = singles.tile([c_in, ncp], bf16)
            W1_s = singles.tile([c_in, c_hid], bf16)
            W2_s = singles.tile([c_hid, c_out], bf16)
            dinv_s = singles.tile([P, T], f32)
            ddiag_s = singles.tile([P, T, P], bf16)
            onesc_s = singles.tile([P, 1], bf16)
            onescf_s = singles.tile([P, 1], f32)
            onesr_s = singles.tile([1, P], bf16)
            gamma_s = singles.tile([1, c_hid], f32)
            beta_s = singles.tile([1, c_hid], f32)
            b2r_s = singles.tile([1, c_out], bf16)
            idxlo_s = singles.tile([P, max(NB_lo, 1) * 8], i16)
            idxhi_s = singles.tile([P, max(NB_hi, 1) * 8], i16)
            gbig_s = singles.tile([P, T, c_hid], bf16)   # AG staging / self rows
            Y_s = singles.tile([P, T, c_hid], bf16)      # BN input (agg1)
            zbig_s = singles.tile([P, T, c_out], f32)
            scsh_s = singles.tile([P, 2 * c_hid], bf16)  # BN scale/shift bcast
            srow_s = singles.tile([1, 2 * c_hid], f32)   # local stat sums
            arres_s = singles.tile([1, 2 * c_hid], f32)  # allreduced sums
            rows_s = singles.tile([1, 8 * c_hid], f32)   # small row scratch
            scshrow_s = singles.tile([1, 2 * c_hid], bf16)

            # ---- internal DRAM (collective bounce) ----
            ag_in1 = dram.tile([ncp, c_hid], bf16)
            ag_out1 = dram.tile([R, c_hid], bf16, addr_space="Shared")
            ag_in2 = dram.tile([ncp, c_hid], bf16)
            ag_out2 = dram.tile([R, c_hid], bf16, addr_space="Shared")
            ar_in = dram.tile([1, 2 * c_hid], f32)
            ar_out = dram.tile([M, 2 * c_hid], f32, addr_space="Shared")

            # ---- load inputs (xT in chunks so P1 starts early) ----
            for dst_t, src_t in ((W1_s, W1_d), (W2_s, W2_d),
                                 (dinv_s, dinv_d), (ddiag_s, ddiag_d),
                                 (onesc_s, onesc_d), (onescf_s, onescf_d),
                                 (onesr_s, onesr_d),
                                 (gamma_s, gamma_d), (beta_s, beta_d),
                                 (b2r_s, b2_d),
                                 (idxlo_s, idxlo_d), (idxhi_s, idxhi_d)):
                nc.sync.dma_start(out=dst_t[:], in_=src_t[:])
            XCH = (T + 3) // 4
            for c0 in range(0, T, XCH):
                c1 = min(c0 + XCH, T)
                nc.sync.dma_start(out=xT_s[:, c0 * P:c1 * P],
                                  in_=xT_d[:, c0 * P:c1 * P])

            # ---- P1: g1 = (x @ W1) * dinv ----
            for t in range(T):
                ph = psum.tile([P, c_hid], f32, tag="pu")
                nc.tensor.matmul(ph[:], lhsT=xT_s[:, t * P:(t + 1) * P],
                                 rhs=W1_s[:], start=True, stop=True)
                nc.scalar.activation(gbig_s[:, t, :], ph[:], AF.Copy,
                                     scale=dinv_s[:, t:t + 1])
                # stage this tile for the AllGather right away
                nc.sync.dma_start(
                    out=ag_in1[t * P:(t + 1) * P, :],
                    in_=gbig_s[:, t, :])

            # ---- P2: AllGather layer-1 table ----
            nc.gpsimd.collective_compute(
                "AllGather", AluOpType.bypass,
                replica_groups=[list(range(M))],
                ins=[ag_in1[:].opt()], outs=[ag_out1[:].opt()])

            ps1 = psum1.tile([1, c_hid], f32)
            ps2 = psum1.tile([1, c_hid], f32)

            # ---- aggregation pass (used for both layers) ----
            # One dma_gather's descriptors must fit the SWDGE ring carveout
            # (~num_idxs/16+1 descs per engine ring), so chunk large gathers.
            GCAP = 8  # blocks per dma_gather (1024 idxs; ring fits ~65 descs/lane)
            qctr = [0]

            def aggregation(ag_out, transposed, epilogue):
                for g in groups:
                    nb_lo, nb_hi = g["nb_lo"], g["nb_hi"]
                    stl = sth = None
                    chunks = {}   # (half, chunk_idx) -> (tile, base_block)
                    for half, off, nb, idx_s, tbl in (
                            (0, g["off_lo"], nb_lo, idxlo_s, ag_out[0:Rh, :]),
                            (1, g["off_hi"], nb_hi, idxhi_s, ag_out[Rh:R, :])):
                        for ci, c0 in enumerate(range(0, nb, GCAP)):
                            sz = min(GCAP, nb - c0)
                            gt = gpool.tile([P, GCAP, c_hid], bf16, tag="gc")
                            nc.gpsimd.dma_gather(
                                gt[:, 0:sz, :], tbl,
                                idx_s[:, (off + c0) * 8:(off + c0 + sz) * 8],
                                num_idxs=sz * P, num_idxs_reg=sz * P,
                                elem_size=c_hid, queue_num=qctr[0] % NQ)
                            qctr[0] += 1
                            chunks[(half, ci)] = (gt, off + c0)
                    if nb_lo:
                        stl = stpool.tile([P, nb_lo, P], bf16, tag="stl")
                        nc.sync.dma_start(
                            out=stl[:],
                            in_=stlo_d[:, g["off_lo"]:g["off_lo"] + nb_lo, :])
                    if nb_hi:
                        sth = stpool.tile([P, nb_hi, P], bf16, tag="sth")
                        nc.sync.dma_start(
                            out=sth[:],
                            in_=sthi_d[:, g["off_hi"]:g["off_hi"] + nb_hi, :])
                    for t in g["tiles"]:
                        pu = psum.tile([P, c_hid], f32, tag="pu")
                        first = True
                        for half, st, off in (
                                (0, stl, g["off_lo"]),
                                (1, sth, g["off_hi"])):
                            for b in tile_blocks[t][half]:
                                gt, base = chunks[(half, (b - off) // GCAP)]
                                if transposed:
                                    nc.tensor.matmul(
                                        pu[:], lhsT=gt[:, b - base, :],
                                        rhs=st[:, b - off, :],
                                        start=first, stop=False)
                                else:
                                    nc.tensor.matmul(
                                        pu[:], lhsT=st[:, b - off, :],
                                        rhs=gt[:, b - base, :],
                                        start=first, stop=False)
                                first = False
                        # self-loop: += diag(dinv)[t] @ g_local[t]
                        if transposed:
                            nc.tensor.matmul(pu[:], lhsT=gbig_s[:, t, :],
                                             rhs=ddiag_s[:, t, :],
                                             start=first, stop=True)
                        else:
                            nc.tensor.matmul(pu[:], lhsT=ddiag_s[:, t, :],
                                             rhs=gbig_s[:, t, :],
                                             start=first, stop=True)
                        epilogue(t, pu)

            # ---- L1 epilogue: Y = pu (dinv already folded), stats ----
            def epi1(t, pu):
                nc.scalar.activation(Y_s[:, t, :], pu[:], AF.Copy)
                sq = scratch.tile([P, c_hid], bf16, tag="sq")
                nc.scalar.activation(sq[:], pu[:], AF.Square)
                nc.tensor.matmul(ps1[:], lhsT=onesc_s[:], rhs=Y_s[:, t, :],
                                 start=(t == 0), stop=(t == T - 1),
                                 skip_group_check=True)
                nc.tensor.matmul(ps2[:], lhsT=onesc_s[:], rhs=sq[:],
                                 start=(t == 0), stop=(t == T - 1),
                                 skip_group_check=True)

            aggregation(ag_out1, False, epi1)

            # ---- P4: BN statistics -> scale/shift ----
            nc.vector.tensor_copy(srow_s[:, 0:c_hid], ps1[:])
            nc.vector.tensor_copy(srow_s[:, c_hid:], ps2[:])
            nc.sync.dma_start(out=ar_in[:], in_=srow_s[:])
            nc.gpsimd.collective_compute(
                "AllGather", AluOpType.bypass,
                replica_groups=[list(range(M))],
                ins=[ar_in[:].opt()], outs=[ar_out[:].opt()])
            arst8_s = singles.tile([M, 2 * c_hid], f32)
            nc.sync.dma_start(out=arst8_s[:], in_=ar_out[:])
            # sum the 8 per-core stat rows via a ones-matmul over partitions
            par = psumaux.tile([P, 2 * c_hid], f32, tag="aux")
            nc.tensor.matmul(par[0:1, :], lhsT=onescf_s[0:M, :],
                             rhs=arst8_s[:], start=True, stop=True)
            nc.vector.tensor_copy(arres_s[:], par[0:1, :])

            H = c_hid
            mean_r = rows_s[:, 0:H]
            e2_r = rows_s[:, H:2 * H]
            var_r = rows_s[:, 2 * H:3 * H]
            rstd_r = rows_s[:, 3 * H:4 * H]
            scale_r = rows_s[:, 4 * H:5 * H]
            tmp_r = rows_s[:, 5 * H:6 * H]
            shift_r = rows_s[:, 6 * H:7 * H]
            nc.vector.tensor_scalar(out=mean_r, in0=arres_s[:, 0:H],
                                    scalar1=1.0 / n, scalar2=None,
                                    op0=AluOpType.mult)
            nc.vector.tensor_scalar(out=e2_r, in0=arres_s[:, H:],
                                    scalar1=1.0 / n, scalar2=None,
                                    op0=AluOpType.mult)
            nc.vector.tensor_tensor(out=var_r, in0=mean_r, in1=mean_r,
                                    op=AluOpType.mult)
            nc.vector.tensor_tensor(out=var_r, in0=e2_r, in1=var_r,
                                    op=AluOpType.subtract)
            nc.vector.tensor_scalar(out=var_r, in0=var_r, scalar1=float(bn_eps),
                                    scalar2=None, op0=AluOpType.add)
            nc.scalar.activation(rstd_r, var_r, AF.Sqrt)
            nc.vector.reciprocal(rstd_r, rstd_r)
            nc.vector.tensor_tensor(out=scale_r, in0=gamma_s[:], in1=rstd_r,
                                    op=AluOpType.mult)
            # bias-before-BN cancels in (y - mean): shift = beta - mean*scale
            nc.vector.tensor_tensor(out=tmp_r, in0=mean_r, in1=scale_r,
                                    op=AluOpType.mult)
            nc.vector.tensor_tensor(out=shift_r, in0=beta_s[:], in1=tmp_r,
                                    op=AluOpType.subtract)
            nc.vector.tensor_copy(scshrow_s[:, 0:H], scale_r)
            nc.vector.tensor_copy(scshrow_s[:, H:], shift_r)
            pbb = psumaux.tile([P, 2 * c_hid], f32, tag="aux")
            nc.tensor.matmul(pbb[:], lhsT=onesr_s[:], rhs=scshrow_s[:],
                             start=True, stop=True)
            nc.scalar.activation(scsh_s[:], pbb[:], AF.Copy)

            # ---- P5: BN apply + ELU + dinv prescale -> g2 (batched) ----
            dinvb_s = singles.tile([P, T], bf16)
            nc.vector.tensor_copy(dinvb_s[:], dinv_s[:])
            PCH = 13
            for c0 in range(0, T, PCH):
                cn = min(PCH, T - c0)
                sc_b = scsh_s[:, 0:H].rearrange(
                    "p (o h) -> p o h", o=1).broadcast_to([P, cn, H])
                sh_b = scsh_s[:, H:].rearrange(
                    "p (o h) -> p o h", o=1).broadcast_to([P, cn, H])
                dv_b = dinvb_s[:, c0:c0 + cn].rearrange(
                    "p (t o) -> p t o", o=1).broadcast_to([P, cn, H])
                z = scratch.tile([P, PCH, c_hid], bf16, tag="z")
                zc = z[:, 0:cn, :]
                nc.vector.tensor_tensor(out=zc, in0=Y_s[:, c0:c0 + cn, :],
                                        in1=sc_b, op=AluOpType.mult)
                nc.vector.tensor_tensor(out=zc, in0=zc, in1=sh_b,
                                        op=AluOpType.add)
                m = scratch.tile([P, PCH, c_hid], bf16, tag="m")
                mc = m[:, 0:cn, :]
                nc.vector.tensor_scalar(out=mc, in0=zc, scalar1=0.0,
                                        scalar2=None, op0=AluOpType.min)
                nc.scalar.activation(mc, mc, AF.Exp)
                nc.vector.tensor_scalar(out=zc, in0=zc, scalar1=0.0,
                                        scalar2=-1.0, op0=AluOpType.max,
                                        op1=AluOpType.add)
                nc.vector.tensor_tensor(out=zc, in0=zc, in1=mc,
                                        op=AluOpType.add)
                nc.vector.tensor_tensor(out=gbig_s[:, c0:c0 + cn, :],
                                        in0=zc, in1=dv_b, op=AluOpType.mult)
                nc.sync.dma_start(
                    out=ag_in2[c0 * P:(c0 + cn) * P, :].rearrange(
                        "(t p) h -> p t h", p=P),
                    in_=gbig_s[:, c0:c0 + cn, :])

            nc.gpsimd.collective_compute(
                "AllGather", AluOpType.bypass,
                replica_groups=[list(range(M))],
                ins=[ag_in2[:].opt()], outs=[ag_out2[:].opt()])

            # ---- L2 epilogue: z = puT^T @ W2 + b2 (transposed aggregation) ----
            def epi2(t, pu):
                A = scratch.tile([P, c_hid], bf16, tag="A")
                nc.scalar.activation(A[:], pu[:], AF.Copy)
                pz = psumaux.tile([P, 2 * c_hid], f32, tag="aux")
                nc.tensor.matmul(pz[:, 0:c_out], lhsT=A[:], rhs=W2_s[:],
                                 start=True, stop=False)
                nc.tensor.matmul(pz[:, 0:c_out], lhsT=onesr_s[:], rhs=b2r_s[:],
                                 start=False, stop=True)
                nc.scalar.activation(zbig_s[:, t, :], pz[:, 0:c_out], AF.Copy)

            aggregation(ag_out2, True, epi2)

            # ---- output ----
            ft = n_per // P
            rem = n_per % P
            if ft:
                nc.sync.dma_start(
                    out=zout_d[0:ft * P, :].rearrange("(t p) f -> p t f", p=P),
                    in_=zbig_s[:, 0:ft, :])
            if rem:
                nc.sync.dma_start(
                    out=zout_d[ft * P:n_per, :],
                    in_=zbig_s[0:rem, ft, :])

    nc.compile()
    return nc


# --------------------------------------------------------------------------
# Input assembly
# --------------------------------------------------------------------------

def make_in_maps(x, W1, b1, gamma, beta, W2, b2, meta, per_core, dinv):
    n_per, T, ncp = meta["n_per"], meta["T"], meta["ncp"]
    c_in = x.shape[1]
    c_hid = W1.shape[1]
    c_out = W2.shape[1]

    ones_col = np.ones((P, 1), dtype=np.float32).astype(BF16)
    ones_row = np.ones((1, P), dtype=np.float32).astype(BF16)
    W1b = np.asarray(W1, np.float32).astype(BF16)
    W2b = np.asarray(W2, np.float32).astype(BF16)

    in_maps = []
    for c in range(M):
        lo, hi = c * n_per, (c + 1) * n_per
        xTc = np.zeros((c_in, ncp), dtype=np.float32)
        xTc[:, :n_per] = np.asarray(x[lo:hi], np.float32).T
        dinv_c = np.zeros(ncp, dtype=np.float32)
        dinv_c[:n_per] = dinv[lo:hi]
        # diag(dinv) per tile, [P, T, P]
        ddiag = np.zeros((P, T, P), dtype=np.float32)
        dd = dinv_c.reshape(T, P)
        for t in range(T):
            ddiag[np.arange(P), t, np.arange(P)] = dd[t]
        pc = per_core[c]
        in_maps.append({
            "xT": xTc.astype(BF16),
            "W1b": W1b, "W2b": W2b,
            "dinv_t": dinv_c.reshape(T, P).T.copy(),
            "ddiag": ddiag.astype(BF16),
            "gamma_r": np.asarray(gamma, np.float32).reshape(1, c_hid),
            "beta_r": np.asarray(beta, np.float32).reshape(1, c_hid),
            "b2_r": np.asarray(b2, np.float32).reshape(1, c_out).astype(BF16),
            "ones_col": ones_col, "ones_row": ones_row,
            "ones_col_f32": np.ones((P, 1), dtype=np.float32),
            "idx_lo": pc["idx_lo"], "idx_hi": pc["idx_hi"],
            "st_lo": pc["st_lo"], "st_hi": pc["st_hi"],
        })
    return in_maps


# --------------------------------------------------------------------------
# Entry point
# --------------------------------------------------------------------------

_CACHE = {}


def _get_compiled(edge_index, n, c_in, c_hid, c_out):
    key = (n, c_in, c_hid, c_out,
           hash(np.asarray(edge_index).tobytes()))
    if key not in _CACHE:
        meta, per_core, dinv = preprocess(edge_index, n)
        nc = build_program(meta, c_in, c_hid, c_out)
        _CACHE[key] = (nc, meta, per_core, dinv)
    return _CACHE[key]


def kernel(x, edge_index, W1, b1, gamma, beta, W2, b2, _trace=False):
    x = np.asarray(x)
    n = x.shape[0]
    nc, meta, per_core, dinv = _get_compiled(
        edge_index, n, x.shape[1], W1.shape[1], W2.shape[1])
    in_maps = make_in_maps(x, W1, b1, gamma, beta, W2, b2,
                           meta, per_core, dinv)
    res = run_bass_kernel_spmd(nc, in_maps, core_ids=list(range(M)),
                               trace=_trace)
    outs = res.results
    z = np.concatenate([outs[c]["zout"] for c in range(M)], axis=0)
    kernel.last_result = res
    return z.astype(np.float32)


# revision 31
# speedup vs baseline: 1.0991x; 1.0991x over previous
"""Two-layer GCN encoder (GCNConv -> BatchNorm -> ELU -> GCNConv) on 8 trn2
NeuronCores.

Sharding: nodes are partitioned across the 8 cores (graph/data parallel).
Each core computes g1 = (x_local @ W1) * dinv (rowwise), the scaled feature
table is AllGathered, and each core aggregates messages for its own
destination nodes by gathering source rows with dma_gather (round-robin over
the 4 SWDGE queues so descriptor generation parallelizes across Q7 core
pairs) and scatter-adding them into PSUM via selection-matrix matmuls.

The selection matrices (one-hot over destination rows, with the dst-side
normalization dinv[d] folded into the nonzeros) are precomputed on the host
and streamed from DRAM via HWDGE - no per-block DVE/ACT work, and no DVE<->
GpSimd SBUF port contention.

Layer 1 aggregates with the selection matrix as lhsT (output [dst, feat]);
BN statistics are AllReduced.  Layer 2 aggregates transposed (selection
matrix as rhs, gathered features as lhsT, output [feat, dst]) so the final
W2 matmul needs no per-tile transpose: z = A2^T @ W2 with b2 folded in as a
rank-1 matmul.

Self-loops are applied as a diag(dinv) matmul per destination tile (the
local table rows are resident in SBUF and already carry one dinv factor).
"""

import numpy as np
import ml_dtypes

import concourse.bass as bass
import concourse.bacc as bacc
import concourse.mybir as mybir
import concourse.tile as tile
from concourse.alu_op_type import AluOpType
from concourse import library_config
from concourse.bass_utils import run_bass_kernel_spmd

P = 128
M = 8  # cores
NQ = 4  # SWDGE gather queues
BF16 = ml_dtypes.bfloat16
FP8 = ml_dtypes.float8_e4m3
AF = mybir.ActivationFunctionType


# --------------------------------------------------------------------------
# Host-side preprocessing
# --------------------------------------------------------------------------

def preprocess(edge_index, n, gt_tiles=4):
    """Sort/partition edges, build per-core padded index arrays, the dense
    selection matrices, and the (uniform across cores) block structure."""
    src = np.asarray(edge_index[0], dtype=np.int64)
    dst = np.asarray(edge_index[1], dtype=np.int64)
    e = src.shape[0]

    n_per = n // M
    assert n_per * M == n
    T = (n_per + P - 1) // P
    ncp = T * P                      # padded nodes per core
    R = M * ncp                      # rows in the gathered table
    Rh = R // 2                      # half split (must fit int16)
    assert Rh < 32768, f"half table {Rh} rows exceeds int16 range"
    assert (4 * ncp) == Rh           # half boundary aligns with core boundary

    deg = np.bincount(dst, minlength=n).astype(np.float64) + 1.0
    dinv = (deg ** -0.5).astype(np.float32)

    # padded global row of each source node
    owner = src // n_per
    r_src = owner * ncp + (src - owner * n_per)          # [e]
    half = (r_src >= Rh).astype(np.int64)
    idx16 = (r_src - half * Rh).astype(np.int64)

    core_of = dst // n_per
    l_dst = dst - core_of * n_per
    t_dst = l_dst // P
    dstloc = l_dst % P

    # per (core, tile, half) edge counts -> uniform block counts
    counts = np.zeros((M, T, 2), dtype=np.int64)
    np.add.at(counts, (core_of, t_dst, half), 1)
    Bth = np.ceil(counts.max(axis=0) / P).astype(np.int64)   # [T, 2]

    # groups of tiles for gathers
    groups = []
    off_lo = off_hi = 0
    for g0 in range(0, T, gt_tiles):
        tiles = list(range(g0, min(g0 + gt_tiles, T)))
        nb_lo = int(sum(Bth[t, 0] for t in tiles))
        nb_hi = int(sum(Bth[t, 1] for t in tiles))
        groups.append(dict(tiles=tiles, off_lo=off_lo, nb_lo=nb_lo,
                           off_hi=off_hi, nb_hi=nb_hi))
        off_lo += nb_lo
        off_hi += nb_hi
    NB_lo, NB_hi = off_lo, off_hi

    # per-tile block index lists (global block number within its half array)
    tile_blocks = []   # [T] -> (lo_block_ids, hi_block_ids)
    blo = bhi = 0
    for t in range(T):
        lo_ids = list(range(blo, blo + int(Bth[t, 0]))); blo += int(Bth[t, 0])
        hi_ids = list(range(bhi, bhi + int(Bth[t, 1]))); bhi += int(Bth[t, 1])
        tile_blocks.append((lo_ids, hi_ids))

    meta = dict(n=n, e=e, n_per=n_per, T=T, ncp=ncp, R=R, Rh=Rh,
                Bth=Bth, groups=groups, tile_blocks=tile_blocks,
                NB_lo=NB_lo, NB_hi=NB_hi)

    # --- per-core arrays ---
    order = np.lexsort((half, t_dst, core_of))   # sort by core, tile, half
    src_s = idx16[order]
    dl_s = dstloc[order]

    per_core = []
    flat_counts = counts  # [M,T,2]
    run_start = np.zeros((M, T, 2), dtype=np.int64)
    np.cumsum(flat_counts.reshape(-1), out=run_start.reshape(-1))
    run_start = run_start - flat_counts  # exclusive prefix

    for c in range(M):
        idx_lo = np.zeros(max(NB_lo, 1) * P, dtype=np.int16)
        idx_hi = np.zeros(max(NB_hi, 1) * P, dtype=np.int16)
        # dense selection matrices, flat over slots
        st_lo = np.zeros((max(NB_lo, 1) * P, P), dtype=np.float32)
        st_hi = np.zeros((max(NB_hi, 1) * P, P), dtype=np.float32)
        for t in range(T):
            for h, (idx_arr, st_arr, blk_ids) in enumerate(
                    ((idx_lo, st_lo, tile_blocks[t][0]),
                     (idx_hi, st_hi, tile_blocks[t][1]))):
                cnt = int(flat_counts[c, t, h])
                s0 = int(run_start[c, t, h])
                if not blk_ids:
                    assert cnt == 0
                    continue
                dst_off = blk_ids[0] * P
                idx_arr[dst_off:dst_off + cnt] = src_s[s0:s0 + cnt]
                slots = np.arange(dst_off, dst_off + cnt)
                st_arr[slots, dl_s[s0:s0 + cnt]] = 1.0
                assert cnt <= len(blk_ids) * P

        def wrap16(a):
            w = a.reshape(-1, 16).T.copy()          # [16, L/16]
            return np.tile(w, (8, 1))               # [128, L/16]

        def part_first(st, nb):
            # [nb*P, P] -> [P(slot), nb, P(dst)]
            return np.ascontiguousarray(
                st.reshape(nb, P, P).transpose(1, 0, 2)).astype(FP8)

        per_core.append(dict(
            idx_lo=wrap16(idx_lo), idx_hi=wrap16(idx_hi),
            st_lo=part_first(st_lo, max(NB_lo, 1)),
            st_hi=part_first(st_hi, max(NB_hi, 1)),
        ))

    return meta, per_core, dinv


# --------------------------------------------------------------------------
# Bass program
# --------------------------------------------------------------------------

def build_program(meta, c_in, c_hid, c_out, bn_eps=1e-5):
    n, T, ncp, R, Rh = meta["n"], meta["T"], meta["ncp"], meta["R"], meta["Rh"]
    n_per = meta["n_per"]
    NB_lo, NB_hi = meta["NB_lo"], meta["NB_hi"]
    groups = meta["groups"]
    tile_blocks = meta["tile_blocks"]
    assert c_in == P and c_hid == P

    f32 = mybir.dt.float32
    bf16 = mybir.dt.bfloat16
    i16 = mybir.dt.int16

    nc = bacc.Bacc(None, target_bir_lowering=False, debug=False, num_devices=M,
                   num_swdge_queues=NQ)

    # ---- I/O ----
    xT_d = nc.declare_dram_parameter("xT", [c_in, ncp], bf16, isOutput=False)
    W1_d = nc.declare_dram_parameter("W1b", [c_in, c_hid], bf16, isOutput=False)
    W2_d = nc.declare_dram_parameter("W2b", [c_hid, c_out], bf16, isOutput=False)
    dinv_d = nc.declare_dram_parameter("dinv_t", [P, T], f32, isOutput=False)
    ident_d = nc.declare_dram_parameter("ident_b", [P, P], bf16, isOutput=False)
    rsq_d = nc.declare_dram_parameter("rsq_t", [1, ncp], bf16, isOutput=False)
    gamma_d = nc.declare_dram_parameter("gamma_r", [1, c_hid], f32, isOutput=False)
    beta_d = nc.declare_dram_parameter("beta_r", [1, c_hid], f32, isOutput=False)
    b2_d = nc.declare_dram_parameter("b2_r", [1, c_out], bf16, isOutput=False)
    onesc_d = nc.declare_dram_parameter("ones_col", [P, 1], bf16, isOutput=False)
    onescf_d = nc.declare_dram_parameter("ones_col_f32", [P, 1], f32, isOutput=False)
    onesr_d = nc.declare_dram_parameter("ones_row", [1, P], bf16, isOutput=False)
    idxlo_d = nc.declare_dram_parameter("idx_lo", [P, max(NB_lo, 1) * 8], i16, isOutput=False)
    idxhi_d = nc.declare_dram_parameter("idx_hi", [P, max(NB_hi, 1) * 8], i16, isOutput=False)
    fp8 = mybir.dt.float8e4
    stlo_d = nc.declare_dram_parameter("st_lo", [P, max(NB_lo, 1), P], fp8, isOutput=False)
    sthi_d = nc.declare_dram_parameter("st_hi", [P, max(NB_hi, 1), P], fp8, isOutput=False)
    zout_d = nc.declare_dram_parameter("zout", [n_per, c_out], f32, isOutput=True)

    nc.gpsimd.load_library(library_config.mlp)

    with tile.TileContext(nc) as tc:
        with (
            tc.tile_pool(name="dram", bufs=1, space="DRAM") as dram,
            tc.tile_pool(name="singles", bufs=1) as singles,
            tc.tile_pool(name="gather", bufs=12) as gpool,
            tc.tile_pool(name="stp", bufs=3) as stpool,
            tc.tile_pool(name="scratch", bufs=2) as scratch,
            tc.tile_pool(name="psum", bufs=3, space="PSUM") as psum,
            tc.tile_pool(name="psumaux", bufs=2, space="PSUM") as psumaux,
            tc.tile_pool(name="psum1", bufs=1, space="PSUM") as psum1,
        ):
            # ---- persistent SBUF ----
            xT_s = singles.tile([c_in, ncp], bf16)
            W1_s = singles.tile([c_in, c_hid], bf16)
            W2_s = singles.tile([c_hid, c_out], bf16)
            dinv_s = singles.tile([P, T], f32)
            ident_s = singles.tile([P, P], bf16)
            rsq_s = singles.tile([1, ncp], bf16)
            onesc_s = singles.tile([P, 1], bf16)
            onescf_s = singles.tile([P, 1], f32)
            onesr_s = singles.tile([1, P], bf16)
            gamma_s = singles.tile([1, c_hid], f32)
            beta_s = singles.tile([1, c_hid], f32)
            b2r_s = singles.tile([1, c_out], bf16)
            idxlo_s = singles.tile([P, max(NB_lo, 1) * 8], i16)
            idxhi_s = singles.tile([P, max(NB_hi, 1) * 8], i16)
            gbig_s = singles.tile([P, T, c_hid], bf16)   # AG staging / self rows
            Y_s = singles.tile([P, T, c_hid], bf16)      # BN input (agg1)
            zbig_s = singles.tile([P, T, c_out], f32)
            scsh_s = singles.tile([P, 2 * c_hid], bf16)  # BN scale/shift bcast
            srow_s = singles.tile([1, 2 * c_hid], f32)   # local stat sums
            arres_s = singles.tile([1, 2 * c_hid], f32)  # allreduced sums
            rows_s = singles.tile([1, 8 * c_hid], f32)   # small row scratch
            scshrow_s = singles.tile([1, 2 * c_hid], bf16)

            # ---- internal DRAM (collective bounce) ----
            ag_in1 = dram.tile([ncp, c_hid], bf16)
            ag_out1 = dram.tile([R, c_hid], bf16, addr_space="Shared")
            ag_in2 = dram.tile([ncp, c_hid], bf16)
            ag_out2 = dram.tile([R, c_hid], bf16, addr_space="Shared")
            ar_in = dram.tile([1, 2 * c_hid], f32)
            ar_out = dram.tile([M, 2 * c_hid], f32, addr_space="Shared")
            warm_in = dram.tile([1, P], bf16)
            warm_out = dram.tile([M, P], bf16, addr_space="Shared")

            # tiny first collective: absorbs the one-time ncfw entry barrier
            # so AG1 isn't gated by it
            nc.gpsimd.collective_compute(
                "AllGather", AluOpType.bypass,
                replica_groups=[list(range(M))],
                ins=[warm_in[:].opt()], outs=[warm_out[:].opt()])

            # ---- load inputs (xT in chunks so P1 starts early) ----
            for dst_t, src_t in ((W1_s, W1_d), (W2_s, W2_d),
                                 (dinv_s, dinv_d), (ident_s, ident_d),
                                 (rsq_s, rsq_d),
                                 (onesc_s, onesc_d), (onescf_s, onescf_d),
                                 (onesr_s, onesr_d),
                                 (gamma_s, gamma_d), (beta_s, beta_d),
                                 (b2r_s, b2_d),
                                 (idxlo_s, idxlo_d), (idxhi_s, idxhi_d)):
                nc.sync.dma_start(out=dst_t[:], in_=src_t[:])
            XCH = (T + 3) // 4
            for c0 in range(0, T, XCH):
                c1 = min(c0 + XCH, T)
                nc.sync.dma_start(out=xT_s[:, c0 * P:c1 * P],
                                  in_=xT_d[:, c0 * P:c1 * P])

            # ---- P1: g1 = (x @ W1) * dinv ----
            for t in range(T):
                ph = psum.tile([P, c_hid], f32, tag="pu")
                nc.tensor.matmul(ph[:], lhsT=xT_s[:, t * P:(t + 1) * P],
                                 rhs=W1_s[:], start=True, stop=True)
                nc.scalar.activation(gbig_s[:, t, :], ph[:], AF.Copy,
                                     scale=dinv_s[:, t:t + 1])
                # stage this tile for the AllGather right away
                nc.sync.dma_start(
                    out=ag_in1[t * P:(t + 1) * P, :],
                    in_=gbig_s[:, t, :])

            # ---- P2: AllGather layer-1 table ----
            nc.gpsimd.collective_compute(
                "AllGather", AluOpType.bypass,
                replica_groups=[list(range(M))],
                ins=[ag_in1[:].opt()], outs=[ag_out1[:].opt()])

            ps1 = psum1.tile([1, c_hid], f32)
            ps2 = psum1.tile([1, c_hid], f32)

            # ---- aggregation pass (used for both layers) ----
            # One dma_gather's descriptors must fit the SWDGE ring carveout
            # (~num_idxs/16+1 descs per engine ring), so chunk large gathers.
            GCAP = 8  # blocks per dma_gather (1024 idxs; ring fits ~65 descs/lane)
            qctr = [0]

            def aggregation(ag_out, transposed, epilogue):
                for g in groups:
                    nb_lo, nb_hi = g["nb_lo"], g["nb_hi"]
                    stl = sth = None
                    chunks = {}   # (half, chunk_idx) -> (tile, base_block)
                    for half, off, nb, idx_s, tbl in (
                            (0, g["off_lo"], nb_lo, idxlo_s, ag_out[0:Rh, :]),
                            (1, g["off_hi"], nb_hi, idxhi_s, ag_out[Rh:R, :])):
                        for ci, c0 in enumerate(range(0, nb, GCAP)):
                            sz = min(GCAP, nb - c0)
                            gt = gpool.tile([P, GCAP, c_hid], bf16, tag="gc")
                            nc.gpsimd.dma_gather(
                                gt[:, 0:sz, :], tbl,
                                idx_s[:, (off + c0) * 8:(off + c0 + sz) * 8],
                                num_idxs=sz * P, num_idxs_reg=sz * P,
                                elem_size=c_hid, queue_num=qctr[0] % NQ)
                            qctr[0] += 1
                            chunks[(half, ci)] = (gt, off + c0)
                    if nb_lo:
                        stl = stpool.tile([P, nb_lo, P], fp8, tag="stl")
                        nc.sync.dma_start(
                            out=stl[:],
                            in_=stlo_d[:, g["off_lo"]:g["off_lo"] + nb_lo, :])
                    if nb_hi:
                        sth = stpool.tile([P, nb_hi, P], fp8, tag="sth")
                        nc.sync.dma_start(
                            out=sth[:],
                            in_=sthi_d[:, g["off_hi"]:g["off_hi"] + nb_hi, :])
                    for t in g["tiles"]:
                        pu = psum.tile([P, c_hid], f32, tag="pu")
                        first = True
                        for half, st, off in (
                                (0, stl, g["off_lo"]),
                                (1, sth, g["off_hi"])):
                            for b in tile_blocks[t][half]:
                                gt, base = chunks[(half, (b - off) // GCAP)]
                                if transposed:
                                    nc.tensor.matmul(
                                        pu[:], lhsT=gt[:, b - base, :],
                                        rhs=st[:, b - off, :],
                                        start=first, stop=False)
                                else:
                                    nc.tensor.matmul(
                                        pu[:], lhsT=st[:, b - off, :],
                                        rhs=gt[:, b - base, :],
                                        start=first, stop=False)
                                first = False
                        # self-loop: += g_local[t] (dinv^2 completes in epi)
                        if transposed:
                            nc.tensor.matmul(pu[:], lhsT=gbig_s[:, t, :],
                                             rhs=ident_s[:],
                                             start=first, stop=True)
                        else:
                            nc.tensor.matmul(pu[:], lhsT=ident_s[:],
                                             rhs=gbig_s[:, t, :],
                                             start=first, stop=True)
                        epilogue(t, pu)

            # ---- L1 epilogue: Y = dinv * pu, stats ----
            def epi1(t, pu):
                nc.scalar.activation(Y_s[:, t, :], pu[:], AF.Copy,
                                     scale=dinv_s[:, t:t + 1])
                sq = scratch.tile([P, c_hid], bf16, tag="sq")
                nc.scalar.activation(sq[:], pu[:], AF.Square,
                                     scale=dinv_s[:, t:t + 1])
                nc.tensor.matmul(ps1[:], lhsT=onesc_s[:], rhs=Y_s[:, t, :],
                                 start=(t == 0), stop=(t == T - 1),
                                 skip_group_check=True)
                nc.tensor.matmul(ps2[:], lhsT=onesc_s[:], rhs=sq[:],
                                 start=(t == 0), stop=(t == T - 1),
                                 skip_group_check=True)

            aggregation(ag_out1, False, epi1)

            # ---- P4: BN statistics -> scale/shift ----
            nc.vector.tensor_copy(srow_s[:, 0:c_hid], ps1[:])
            nc.vector.tensor_copy(srow_s[:, c_hid:], ps2[:])
            nc.sync.dma_start(out=ar_in[:], in_=srow_s[:])
            nc.gpsimd.collective_compute(
                "AllGather", AluOpType.bypass,
                replica_groups=[list(range(M))],
                ins=[ar_in[:].opt()], outs=[ar_out[:].opt()])
            arst8_s = singles.tile([M, 2 * c_hid], f32)
            nc.sync.dma_start(out=arst8_s[:], in_=ar_out[:])
            # sum the 8 per-core stat rows via a ones-matmul over partitions
            par = psumaux.tile([P, 2 * c_hid], f32, tag="aux")
            nc.tensor.matmul(par[0:1, :], lhsT=onescf_s[0:M, :],
                             rhs=arst8_s[:], start=True, stop=True)
            nc.vector.tensor_copy(arres_s[:], par[0:1, :])

            H = c_hid
            mean_r = rows_s[:, 0:H]
            e2_r = rows_s[:, H:2 * H]
            var_r = rows_s[:, 2 * H:3 * H]
            rstd_r = rows_s[:, 3 * H:4 * H]
            scale_r = rows_s[:, 4 * H:5 * H]
            tmp_r = rows_s[:, 5 * H:6 * H]
            shift_r = rows_s[:, 6 * H:7 * H]
            nc.vector.tensor_scalar(out=mean_r, in0=arres_s[:, 0:H],
                                    scalar1=1.0 / n, scalar2=None,
                                    op0=AluOpType.mult)
            nc.vector.tensor_scalar(out=e2_r, in0=arres_s[:, H:],
                                    scalar1=1.0 / n, scalar2=None,
                                    op0=AluOpType.mult)
            nc.vector.tensor_tensor(out=var_r, in0=mean_r, in1=mean_r,
                                    op=AluOpType.mult)
            nc.vector.tensor_tensor(out=var_r, in0=e2_r, in1=var_r,
                                    op=AluOpType.subtract)
            nc.vector.tensor_scalar(out=var_r, in0=var_r, scalar1=float(bn_eps),
                                    scalar2=None, op0=AluOpType.add)
            nc.scalar.activation(rstd_r, var_r, AF.Sqrt)
            nc.vector.reciprocal(rstd_r, rstd_r)
            nc.vector.tensor_tensor(out=scale_r, in0=gamma_s[:], in1=rstd_r,
                                    op=AluOpType.mult)
            # bias-before-BN cancels in (y - mean): shift = beta - mean*scale
            nc.vector.tensor_tensor(out=tmp_r, in0=mean_r, in1=scale_r,
                                    op=AluOpType.mult)
            nc.vector.tensor_tensor(out=shift_r, in0=beta_s[:], in1=tmp_r,
                                    op=AluOpType.subtract)
            nc.vector.tensor_copy(scshrow_s[:, 0:H], scale_r)
            nc.vector.tensor_copy(scshrow_s[:, H:], shift_r)
            pbb = psumaux.tile([P, 2 * c_hid], f32, tag="aux")
            nc.tensor.matmul(pbb[:], lhsT=onesr_s[:], rhs=scshrow_s[:],
                             start=True, stop=True)
            nc.scalar.activation(scsh_s[:], pbb[:], AF.Copy)

            # ---- P5: BN apply + ELU + dinv prescale -> g2 (batched) ----
            dinvb_s = singles.tile([P, T], bf16)
            nc.vector.tensor_copy(dinvb_s[:], dinv_s[:])
            PCH = 13
            for c0 in range(0, T, PCH):
                cn = min(PCH, T - c0)
                sc_b = scsh_s[:, 0:H].rearrange(
                    "p (o h) -> p o h", o=1).broadcast_to([P, cn, H])
                sh_b = scsh_s[:, H:].rearrange(
                    "p (o h) -> p o h", o=1).broadcast_to([P, cn, H])
                dv_b = dinvb_s[:, c0:c0 + cn].rearrange(
                    "p (t o) -> p t o", o=1).broadcast_to([P, cn, H])
                z = scratch.tile([P, PCH, c_hid], bf16, tag="z")
                zc = z[:, 0:cn, :]
                nc.vector.tensor_tensor(out=zc, in0=Y_s[:, c0:c0 + cn, :],
                                        in1=sc_b, op=AluOpType.mult)
                nc.vector.tensor_tensor(out=zc, in0=zc, in1=sh_b,
                                        op=AluOpType.add)
                m = scratch.tile([P, PCH, c_hid], bf16, tag="m")
                mc = m[:, 0:cn, :]
                nc.vector.tensor_scalar(out=mc, in0=zc, scalar1=0.0,
                                        scalar2=None, op0=AluOpType.min)
                nc.scalar.activation(mc, mc, AF.Exp)
                nc.vector.tensor_scalar(out=zc, in0=zc, scalar1=0.0,
                                        scalar2=-1.0, op0=AluOpType.max,
                                        op1=AluOpType.add)
                nc.vector.tensor_tensor(out=zc, in0=zc, in1=mc,
                                        op=AluOpType.add)
                nc.vector.tensor_tensor(out=gbig_s[:, c0:c0 + cn, :],
                                        in0=zc, in1=dv_b, op=AluOpType.mult)
                nc.sync.dma_start(
                    out=ag_in2[c0 * P:(c0 + cn) * P, :].rearrange(
                        "(t p) h -> p t h", p=P),
                    in_=gbig_s[:, c0:c0 + cn, :])

            nc.gpsimd.collective_compute(
                "AllGather", AluOpType.bypass,
                replica_groups=[list(range(M))],
                ins=[ag_in2[:].opt()], outs=[ag_out2[:].opt()])

            # ---- L2 epilogue: z = dinv * (puT^T @ W2 + sqrt(deg) b2) ----
            def epi2(t, pu):
                A = scratch.tile([P, c_hid], bf16, tag="A")
                nc.scalar.activation(A[:], pu[:], AF.Copy)
                pz = psumaux.tile([P, 2 * c_hid], f32, tag="aux")
                nc.tensor.matmul(pz[:, 0:c_out], lhsT=A[:], rhs=W2_s[:],
                                 start=True, stop=False)
                nc.tensor.matmul(pz[:, 0:c_out],
                                 lhsT=rsq_s[:, t * P:(t + 1) * P],
                                 rhs=b2r_s[:], start=False, stop=True)
                nc.scalar.activation(zbig_s[:, t, :], pz[:, 0:c_out], AF.Copy,
                                     scale=dinv_s[:, t:t + 1])

            aggregation(ag_out2, True, epi2)

            # ---- output ----
            ft = n_per // P
            rem = n_per % P
            if ft:
                nc.sync.dma_start(
                    out=zout_d[0:ft * P, :].rearrange("(t p) f -> p t f", p=P),
                    in_=zbig_s[:, 0:ft, :])
            if rem:
                nc.sync.dma_start(
                    out=zout_d[ft * P:n_per, :],
                    in_=zbig_s[0:rem, ft, :])

    nc.compile()
    return nc


# --------------------------------------------------------------------------
# Input assembly
# --------------------------------------------------------------------------

def make_in_maps(x, W1, b1, gamma, beta, W2, b2, meta, per_core, dinv):
    n_per, T, ncp = meta["n_per"], meta["T"], meta["ncp"]
    c_in = x.shape[1]
    c_hid = W1.shape[1]
    c_out = W2.shape[1]

    ones_col = np.ones((P, 1), dtype=np.float32).astype(BF16)
    ones_row = np.ones((1, P), dtype=np.float32).astype(BF16)
    W1b = np.asarray(W1, np.float32).astype(BF16)
    W2b = np.asarray(W2, np.float32).astype(BF16)

    in_maps = []
    for c in range(M):
        lo, hi = c * n_per, (c + 1) * n_per
        xTc = np.zeros((c_in, ncp), dtype=np.float32)
        xTc[:, :n_per] = np.asarray(x[lo:hi], np.float32).T
        dinv_c = np.zeros(ncp, dtype=np.float32)
        dinv_c[:n_per] = dinv[lo:hi]
        rsq_c = np.zeros(ncp, dtype=np.float32)
        rsq_c[:n_per] = 1.0 / dinv[lo:hi]
        pc = per_core[c]
        in_maps.append({
            "xT": xTc.astype(BF16),
            "W1b": W1b, "W2b": W2b,
            "dinv_t": dinv_c.reshape(T, P).T.copy(),
            "ident_b": np.eye(P, dtype=np.float32).astype(BF16),
            "rsq_t": rsq_c.reshape(1, ncp).astype(BF16),
            "gamma_r": np.asarray(gamma, np.float32).reshape(1, c_hid),
            "beta_r": np.asarray(beta, np.float32).reshape(1, c_hid),
            "b2_r": np.asarray(b2, np.float32).reshape(1, c_out).astype(BF16),
            "ones_col": ones_col, "ones_row": ones_row,
            "ones_col_f32": np.ones((P, 1), dtype=np.float32),
            "idx_lo": pc["idx_lo"], "idx_hi": pc["idx_hi"],
            "st_lo": pc["st_lo"], "st_hi": pc["st_hi"],
        })
    return in_maps


# --------------------------------------------------------------------------
# Entry point
# --------------------------------------------------------------------------

_CACHE = {}


def _get_compiled(edge_index, n, c_in, c_hid, c_out):
    key = (n, c_in, c_hid, c_out,
           hash(np.asarray(edge_index).tobytes()))
    if key not in _CACHE:
        meta, per_core, dinv = preprocess(edge_index, n)
        nc = build_program(meta, c_in, c_hid, c_out)
        _CACHE[key] = (nc, meta, per_core, dinv)
    return _CACHE[key]


def kernel(x, edge_index, W1, b1, gamma, beta, W2, b2, _trace=False):
    x = np.asarray(x)
    n = x.shape[0]
    nc, meta, per_core, dinv = _get_compiled(
        edge_index, n, x.shape[1], W1.shape[1], W2.shape[1])
    in_maps = make_in_maps(x, W1, b1, gamma, beta, W2, b2,
                           meta, per_core, dinv)
    res = run_bass_kernel_spmd(nc, in_maps, core_ids=list(range(M)),
                               trace=_trace)
    outs = res.results
    z = np.concatenate([outs[c]["zout"] for c in range(M)], axis=0)
    kernel.last_result = res
    return z.astype(np.float32)
